# revision 16
# baseline (speedup 1.0000x reference)
"""MoE routing kernel (nn_Bool_40793599377512) for 8 trn2 NeuronCores.

out[n] = tanh(x[n] @ W[g(n)] + b[g(n)]),  g(n) = (mean(x[n]) > 0)

Strategy (expert-parallel): route rows on the host (cheap: one mean per
row), give each core a slice of rows that all use ONE expert, and run a
dense  y.T = W_e.T @ x_slice.T  matmul per core.

Mixed-precision split of the K=4096 contraction:
 - 28 k-tiles "clean": x moving in bf16, W stationary in bf16 at 1
   col/cycle on the PE.
 - 4 k-tiles "noisy": both operands fp8-e4m3, computed with DoubleRow
   matmuls that contract K=256 per instruction (2 k-tiles at 1
   col/cycle) -- 6.25% fewer PE cycles overall for ~1.3e-2 extra
   relative error (gate is 2e-2; measured end-to-end 1.47e-2).
All products share one fixed-point scale F = (32 * 2048): clean W is
pre-scaled by F in bf16, noisy x/W carry 32/2048 into fp8, and the
final tanh activation applies 1/F. y is written back as bf16.

W is pre-blocked on the host so every W DMA is a fully contiguous
per-partition transfer. Rows are padded per-core to a fixed capacity so
the compiled program is input-independent.
"""

import functools
import os
import sys
from contextlib import ExitStack

import ml_dtypes
import numpy as np

for _p in ("/opt/trn_rl_repo", "/root/.axon_site/_ro/trn_rl_repo"):
    if os.path.isdir(_p) and _p not in sys.path:
        sys.path.append(_p)

import concourse.bacc as bacc
import concourse.tile as tile
from concourse import mybir
from concourse.bass_utils import run_bass_kernel_spmd


def _ensure_axon_ntff_hook():
    """Register the NTFF-profile hook that bass_utils expects under axon.

    This image's ``antenv`` package lacks ``axon_hooks``; without it,
    ``run_bass_kernel_spmd(trace=True)`` (e.g. via BASS_TRACE=1) crashes
    on import instead of profiling. Provide the module and wire in the
    ctypes hook from the axon boot shim when available.
    """
    try:
        import antenv.axon_hooks  # noqa: F401

        return
    except ImportError:
        pass
    try:
        import types

        import antenv

        mod = types.ModuleType("antenv.axon_hooks")
        state = {"hook": None}
        mod.set_axon_ntff_profile_hook = lambda h: state.__setitem__("hook", h)
        mod.get_axon_ntff_profile_hook = lambda: state["hook"]
        sys.modules["antenv.axon_hooks"] = mod
        antenv.axon_hooks = mod
        if "/root/.axon_site" not in sys.path:
            sys.path.append("/root/.axon_site")
        from trn_agent_boot.trn_boot import _ntff_profile_via_ctypes

        hook = _ntff_profile_via_ctypes("/opt/axon/libaxon_pjrt.so")
        if hook is not None:
            mod.set_axon_ntff_profile_hook(hook)
    except Exception:
        pass


_ensure_axon_ntff_hook()

N_TOK, D_IN, D_OUT, N_EXPERTS, NCORES = 8192, 4096, 4096, 2, 8
P = 128
F32 = mybir.dt.float32
BF16 = mybir.dt.bfloat16
F8E4 = mybir.dt.float8e4
E4NP = ml_dtypes.float8_e4m3fn
BFNP = ml_dtypes.bfloat16
DR = mybir.MatmulPerfMode.DoubleRow

KCT = 26           # clean k-tiles (bf16)
KC = KCT * P       # clean K region of d_in
NPAIR = 3          # fp8 DoubleRow pairs covering k-tiles 26..31
N_SEG = 2          # a clean W column is fetched as 2 K-segments
SEG_K = KCT // N_SEG

SX8, SW8 = 32.0, 2048.0   # fp8 operand scales
F_SCALE = SX8 * SW8       # common product scale

LAST_RUN = None  # BassKernelResults of the most recent hardware run


def _chunks(c):
    """Split token count c into balanced matmul N-chunks (<=512 each)."""
    n = -(-c // 512)
    q, tail = divmod(c, 8)
    units = [q // n + (1 if j < q % n else 0) for j in range(n)]
    out = [8 * u for j, u in enumerate(units)]
    out[-1] += tail  # c is snapped to 8 in practice, so tail == 0
    return out


def _subchunks(ch):
    """DoubleRow moving free dim is capped at 2*256: split a chunk."""
    return [(0, min(ch, 256))] + ([(256, ch - 256)] if ch > 256 else [])


@functools.lru_cache(maxsize=4)
def _build(c_cap, d_in=D_IN, d_out=D_OUT):
    """Build + compile the per-core Bass program (same for all 8 cores).

    Inputs per core:
      xT  [KC, c_cap]  bf16 -- clean x, transposed
      x8  [NPAIR, P, 2, c_cap] e4m3 -- noisy x pairs (slot-interleaved)
      W   [mt, N_SEG, P, SEG_K*P] bf16 -- clean W * F, pre-blocked
      W8  [mt, P, NPAIR*2*P] e4m3 -- noisy W pairs per m-column
      bT  [P, mt] f32
    Output: yT [d_out, c_cap] bf16.
    """
    mt = d_out // P
    chunks = _chunks(c_cap)

    nc = bacc.Bacc(
        "TRN2", target_bir_lowering=False, debug=False, num_devices=NCORES
    )
    xT = nc.dram_tensor("xT", [KC, c_cap], BF16, kind="ExternalInput").ap()
    x8d = nc.dram_tensor(
        "x8", [NPAIR, P, 2, c_cap], F8E4, kind="ExternalInput"
    ).ap()
    Wd = nc.dram_tensor(
        "W", [mt, N_SEG, P, SEG_K * P], BF16, kind="ExternalInput"
    ).ap()
    W8d = nc.dram_tensor(
        "W8", [mt, P, NPAIR * 2 * P], F8E4, kind="ExternalInput"
    ).ap()
    bd = nc.dram_tensor("bT", [P, mt], F32, kind="ExternalInput").ap()
    yT = nc.dram_tensor("yT", [d_out, c_cap], BF16, kind="ExternalOutput").ap()

    w_bufs = 6

    n_ch = len(chunks)
    ps_bufs = [8 // n_ch + (1 if j < 8 % n_ch else 0) for j in range(n_ch)]
    ps_bufs = [min(b, 4) for b in ps_bufs]

    # After these clean k indices, one noisy (pair, chunk) column-pass is
    # inserted; spacing them 3 clean k-tiles apart keeps their weight
    # loads hidden under adjacent clean matmuls.
    noisy_slot = {
        1 + 3 * i: (pr, j)
        for i, (pr, j) in enumerate(
            (pr, j) for pr in range(NPAIR) for j in range(n_ch)
        )
    }

    with tile.TileContext(nc) as tc:
        with ExitStack() as ctx:
            xt_pool = ctx.enter_context(tc.tile_pool(name="xt", bufs=1))
            x8_pool = ctx.enter_context(tc.tile_pool(name="x8", bufs=1))
            w_pool = ctx.enter_context(
                tc.tile_pool(name="w", bufs=w_bufs * (N_SEG + 1))
            )
            ps_pool = ctx.enter_context(
                tc.tile_pool(name="ps", bufs=1, space="PSUM")
            )
            out_pool = ctx.enter_context(tc.tile_pool(name="out", bufs=2))
            b_pool = ctx.enter_context(tc.tile_pool(name="b", bufs=1))
            warm_pool = ctx.enter_context(tc.tile_pool(name="warm", bufs=1))

            xt_all = xt_pool.tile([P, KCT * c_cap], BF16)
            x8_all = x8_pool.tile([P, NPAIR * 2 * c_cap], F8E4)
            x8v = x8_all[:].rearrange(
                "p (pr s c) -> p pr s c", pr=NPAIR, s=2
            )

            # Dependency-free dummy matmuls fill the PE during the
            # initial DMA fill so the clock has ramped when the first
            # real matmul issues. They rotate through the ps0 buffers;
            # start=True resets accumulation so the garbage never
            # reaches a real result.
            warm_t = warm_pool.tile([P, 256], BF16)
            nc.vector.memset(warm_t[:], 0.0)
            for i in range(26):
                warm_ps = ps_pool.tile(
                    [P, chunks[0]],
                    F32,
                    tag="ps0",
                    name=f"warm_{i}",
                    bufs=ps_bufs[0],
                )
                nc.tensor.matmul(
                    warm_ps[:, :256],
                    warm_t[:, :P],
                    warm_t[:],
                    start=True,
                    stop=True,
                )

            def load_w(m):
                segs = []
                for s in range(N_SEG):
                    wt = w_pool.tile(
                        [P, SEG_K * P], BF16, name=f"wt{m}_{s}", tag="wt"
                    )
                    nc.sync.dma_start(wt[:], Wd[m, s])
                    segs.append(wt)
                w8t = w_pool.tile(
                    [P, NPAIR * 2 * P], F8E4, name=f"w8{m}", tag="w8"
                )
                nc.sync.dma_start(w8t[:], W8d[m])
                return segs, w8t

            def load_xt(k):
                nc.sync.dma_start(
                    xt_all[:, k * c_cap : (k + 1) * c_cap],
                    xT[k * P : (k + 1) * P, :],
                )

            def load_x8(pr):
                nc.sync.dma_start(x8v[:, pr], x8d[pr])

            # Startup: x k0 + fp8 x + W col0 first (PE can begin at
            # ~1us of data), then stream the remaining x k-tiles
            # interleaved with the next W columns.
            bias_t = b_pool.tile([P, mt], F32)
            nc.sync.dma_start(bias_t[:], bd)
            w_head = min(w_bufs, mt)
            load_xt(0)
            for pr in range(NPAIR):
                load_x8(pr)
            wts = {0: load_w(0)}
            xk = 1
            while xk < 4:
                load_xt(xk)
                xk += 1
            wts[1] = load_w(1)
            while xk < KCT:
                load_xt(xk)
                xk += 1
            for i in range(2, w_head):
                wts[i] = load_w(i)

            offs = [sum(chunks[:j]) for j in range(n_ch)]

            def mm_clean(psum, wsegs, k, off, ch, start, stop):
                nc.tensor.matmul(
                    psum[:],
                    wsegs[k // SEG_K][
                        :, (k % SEG_K) * P : (k % SEG_K + 1) * P
                    ],
                    xt_all[:, k * c_cap + off : k * c_cap + off + ch],
                    start=start,
                    stop=stop,
                )

            def mm_noisy(psum, w8v_m, pr, j):
                for so, chn in _subchunks(chunks[j]):
                    a = offs[j] + so
                    nc.tensor.matmul(
                        psum[:, so : so + chn],
                        w8v_m[:, pr],
                        x8v[:, pr, :, a : a + chn],
                        start=False,
                        stop=False,
                        perf_mode=DR,
                    )

            def act_dma(m, j, psum, out_t):
                off, ch = offs[j], chunks[j]
                nc.scalar.activation(
                    out_t[:, off : off + ch],
                    psum[:],
                    mybir.ActivationFunctionType.Tanh,
                    bias=bias_t[:, m : m + 1],
                    scale=float(1.0 / F_SCALE),
                )
                nc.sync.dma_start(
                    yT[m * P : (m + 1) * P, off : off + ch],
                    out_t[:, off : off + ch],
                )

            def ps_tile(m, j):
                return ps_pool.tile(
                    [P, chunks[j]],
                    F32,
                    tag=f"ps{j}",
                    name=f"ps{j}_{m}",
                    bufs=ps_bufs[j],
                )

            for m in range(mt - 1):
                (wsegs, w8t) = wts.pop(m) if m in wts else load_w(m)
                w8v_m = w8t[:].rearrange("p (pr s c) -> p pr s c", pr=NPAIR, s=2)
                psums = [ps_tile(m, j) for j in range(n_ch)]
                for k in range(KCT):
                    for j, ch in enumerate(chunks):
                        mm_clean(
                            psums[j], wsegs, k, offs[j], ch,
                            start=(k == 0), stop=(k == KCT - 1),
                        )
                    if k in noisy_slot:
                        pr, j = noisy_slot[k]
                        mm_noisy(psums[j], w8v_m, pr, j)
                out_t = out_pool.tile([P, c_cap], BF16)
                for j in range(n_ch):
                    act_dma(m, j, psums[j], out_t)

            # Last column: j-outer k-sweeps so the first chunks' tanh +
            # output DMA overlap the PE finishing the later chunks.
            m = mt - 1
            (wsegs, w8t) = wts.pop(m) if m in wts else load_w(m)
            w8v_m = w8t[:].rearrange("p (pr s c) -> p pr s c", pr=NPAIR, s=2)
            out_t = out_pool.tile([P, c_cap], BF16)
            for j, ch in enumerate(chunks):
                psum = ps_tile(m, j)
                for k in range(KCT):
                    mm_clean(
                        psum, wsegs, k, offs[j], ch,
                        start=(k == 0), stop=(k == KCT - 1),
                    )
                    if k >= 2 and (k - 2) % 3 == 0 and (k - 2) // 3 < NPAIR:
                        mm_noisy(psum, w8v_m, (k - 2) // 3, j)
                act_dma(m, j, psum, out_t)
    nc.compile()
    return nc


def _route(x):
    """Expert id per row, matching the reference's (mean(x,-1) > 0)."""
    # float64 accumulation: any fp32 summation order agrees with this
    # sign unless |mean| is within ~1e-9 of zero (never for randn data).
    return (x.astype(np.float64).mean(axis=1) > 0.0).astype(np.int32)


def _core_assignment(counts):
    """Number of cores per expert minimizing the max per-core row load."""
    best = None
    for c0 in range(NCORES + 1):
        c1 = NCORES - c0
        if (counts[0] > 0 and c0 == 0) or (counts[1] > 0 and c1 == 0):
            continue
        load = 0
        if c0:
            load = max(load, -(-counts[0] // c0))
        if c1:
            load = max(load, -(-counts[1] // c1))
        if best is None or load < best[0]:
            best = (load, c0, c1)
    return best


def _prep_w(We):
    """Split one expert's [d_in, d_out] f32 weights into the clean bf16
    pre-blocked tensor (scaled by F) and the noisy e4m3 pair tensor."""
    d_in, d_out = We.shape
    mt = d_out // P
    Wc = (We[:KC] * F_SCALE).astype(BFNP)
    Wc = Wc.reshape(N_SEG, SEG_K, P, mt, P)
    Wc = np.ascontiguousarray(Wc.transpose(3, 0, 2, 1, 4)).reshape(
        mt, N_SEG, P, SEG_K * P
    )
    W8 = (We[KC:] * SW8).astype(E4NP)
    W8 = W8.reshape(NPAIR, 2, P, mt, P)
    W8 = np.ascontiguousarray(W8.transpose(3, 2, 0, 1, 4)).reshape(
        mt, P, NPAIR * 2 * P
    )
    return Wc, W8


def kernel(x, W, b):
    global LAST_RUN
    x = np.ascontiguousarray(x, dtype=np.float32)
    W = np.ascontiguousarray(W, dtype=np.float32)
    b = np.ascontiguousarray(b, dtype=np.float32)
    n_tok, d_in = x.shape
    d_out = W.shape[2]
    mt = d_out // P

    g = _route(x)
    idx = [np.nonzero(g == e)[0] for e in range(N_EXPERTS)]
    load, c0, c1 = _core_assignment([len(idx[0]), len(idx[1])])
    c_cap = max(256, -(-load // 8) * 8)

    nc = _build(c_cap, d_in, d_out)

    # Quantize x once, then gather per-core column slices from the
    # transposed copies.
    xcT = np.ascontiguousarray(x[:, :KC].astype(BFNP).T)       # [KC, n]
    xnT = np.ascontiguousarray((x[:, KC:] * SX8).astype(E4NP).T)  # [4P, n]

    groups = []  # per core: (expert, row-index array)
    for e, ncr in ((0, c0), (1, c1)):
        if ncr:
            groups.extend((e, part) for part in np.array_split(idx[e], ncr))
    assert len(groups) == NCORES

    Wprep = [_prep_w(W[e]) for e in range(N_EXPERTS)]
    bT = [np.ascontiguousarray(b[e].reshape(mt, P).T) for e in range(N_EXPERTS)]
    in_maps = []
    for e, rows in groups:
        xTc = np.zeros((KC, c_cap), dtype=BFNP)
        xn = np.zeros((NPAIR * 2 * P, c_cap), dtype=E4NP)
        if len(rows):
            np.take(xcT, rows, axis=1, out=xTc[:, : len(rows)])
            np.take(xnT, rows, axis=1, out=xn[:, : len(rows)])
        # [(pr s p), c] -> [pr, p, s, c]
        x8 = np.ascontiguousarray(
            xn.reshape(NPAIR, 2, P, c_cap).transpose(0, 2, 1, 3)
        )
        in_maps.append(
            {
                "xT": xTc,
                "x8": x8,
                "W": Wprep[e][0],
                "W8": Wprep[e][1],
                "bT": bT[e],
            }
        )

    res = run_bass_kernel_spmd(nc, in_maps, core_ids=list(range(NCORES)))
    LAST_RUN = res

    y = np.empty((n_tok, d_out), dtype=np.float32)
    for (e, rows), core_out in zip(groups, res.results):
        if len(rows):
            y[rows] = core_out["yT"][:, : len(rows)].T.astype(np.float32)
    return y
